# revision 27
# baseline (speedup 1.0000x reference)
"""Trainium2 Bass kernel for GQA self-attention (non-causal, RoPE).

Reference computation (B=2, T=2048, C=2048, 16 q-heads, 4 kv-heads, d=128):
    q = x @ Wq.T ; k = x @ Wk.T ; v = x @ Wv.T
    q, k <- RoPE(q, k)
    att = softmax(q k^T / sqrt(d))        (no causal mask)
    out = att @ v ; y = out @ Wo.T
Sharding: 8 cores = DP(batch)=2 x TP(kv-head group)=4; host sums the four
o-proj partials per batch element.

v2 design (all matmul operands bf16; fp32 PSUM accumulation):
- bf16 halves every DMA and keeps PE at 1 cycle/row; rel-err budget 2e-2 vs
  ~1e-3 for bf16 end-to-end.
- Host pre-arranges every tensor into its exact SBUF layout so each DMA is a
  contiguous >=512B-per-descriptor copy at full modeled bandwidth, emitted in
  k-slice pairs interleaved with x chunks so the first matmul starts ~3us in
  (the v1 ordering serialized ~14us of weight DMA ahead of the first x tile).
- RoPE runs entirely off the PE: ACT/DVE/Pool drain the projection PSUM,
  Pool does the cos/sin products (half-rotation via partition-offset reads),
  DVE does the final add, writing bf16 qT/kT.
- v is transposed [d,t]->[t,d] by the DMA XBAR (16-bit transpose, 14ns/tile)
  instead of PE transposes.
- Softmax row-sums: Pool+DVE in-place pairwise tree over the 16 et tiles,
  then ONE ones-matmul per (head, s-chunk) instead of five (PE 41k->8k
  cycles). The matmul+recip+normalize for block i are emitted inside block
  i+1 so the PE never waits on the tree.
- o-proj writes bf16 y via ACT copy, per-512-column DMAs; host upcasts+sums.
"""

import numpy as np

B = 2
T = 2048
C = 2048
HD = 128
N_HEAD = 16
N_KV = 4
KV_REP = N_HEAD // N_KV
ROPE_THETA = 10000.0
NCORES = 8
TP = 4
SCALE = 1.0 / float(np.sqrt(HD))

TCH = 512  # token chunk
NT = T // 128  # 16 token tiles
NCH = T // TCH  # 4 token chunks
NKC = C // 128  # 16 contraction k-slices

_CACHE = {}


def _build_nc():
    import concourse.bass as bass
    import concourse.mybir as mybir
    import concourse.tile as tile
    from concourse import bacc

    f32 = mybir.dt.float32
    bf16 = mybir.dt.bfloat16
    Exp = mybir.ActivationFunctionType.Exp

    nc = bacc.Bacc(None)

    xd = nc.declare_dram_parameter("xd", [128, NCH, NKC, TCH], bf16, isOutput=False)
    wqd = nc.declare_dram_parameter("wqd", [128, NKC, 4 * HD], bf16, isOutput=False)
    wkd = nc.declare_dram_parameter("wkd", [128, NKC, HD], bf16, isOutput=False)
    wvd = nc.declare_dram_parameter("wvd", [128, NKC, HD], bf16, isOutput=False)
    wod = nc.declare_dram_parameter("wod", [128, 4, C], bf16, isOutput=False)
    cosd = nc.declare_dram_parameter("cosd", [HD, T], bf16, isOutput=False)
    sind = nc.declare_dram_parameter("sind", [HD, T], bf16, isOutput=False)
    onesd = nc.declare_dram_parameter("ones", [128, 128], bf16, isOutput=False)
    y = nc.declare_dram_parameter("y", [T, C], bf16, isOutput=True)

    with tile.TileContext(nc) as tc:
        with (
            tc.tile_pool(name="persist", bufs=1) as persist,
            # B/C-phase SBUF pools live OUTSIDE the phase-A pool scope: the
            # stack allocator then gives them addresses disjoint from A's
            # pools, so phase B's first writes don't serialize on A's
            # last consumers draining (6.7us A->B stall otherwise)
            tc.tile_pool(name="epool", bufs=2) as epool,
            tc.tile_pool(name="rpool", bufs=2) as rpool,
            tc.tile_pool(name="ypool", bufs=4) as ypool,
        ):
            qT_sb = persist.tile([128, 4, T], bf16)  # [d, qhead, t]
            kT_sb = persist.tile([128, T], bf16)  # [d, t]
            v_sb = persist.tile([128, NT, HD], bf16)  # [t%128, tblk, d]
            outT_sb = persist.tile([128, 4, T], bf16)  # [d, qhead, s]
            ones_sb = persist.tile([128, 128], bf16)
            wo_sb = persist.tile([128, 4, C], bf16)

            # ---------------- Phase A: projections + RoPE ----------------
            with (
                tc.tile_pool(name="cossin", bufs=1) as cossin,
                tc.tile_pool(name="wA", bufs=1) as wA,
                tc.tile_pool(name="xload", bufs=4) as xload,
                tc.tile_pool(name="ropet", bufs=1) as ropet,
                tc.tile_pool(name="ppq01", bufs=2, space="PSUM") as ppq01,
                tc.tile_pool(name="ppA", bufs=1, space="PSUM") as ppA,
            ):
                cos_sb = cossin.tile([128, T], bf16)
                sin_sb = cossin.tile([128, T], bf16)
                wq_sb = wA.tile([128, NKC, 4 * HD], bf16)
                wk_sb = wA.tile([128, NKC, HD], bf16)
                wv_sb = wA.tile([128, NKC, HD], bf16)
                warm = wA.tile([128, 1], f32)
                qa_pool = [
                    ropet.tile([128, TCH], bf16, name=f"qa{i}") for i in range(6)
                ]
                m_pool = [ropet.tile([128, TCH], bf16, name=f"m{i}") for i in range(4)]
                t2_pool = [
                    ropet.tile([128, TCH], bf16, name=f"t2{i}") for i in range(4)
                ]
                vt_pool = [
                    ropet.tile([128, TCH], bf16, name=f"vt{i}") for i in range(2)
                ]
                qs_pool = [
                    ropet.tile([128, TCH], bf16, name=f"qs{i}") for i in range(2)
                ]
                SWAP_MASK = [i ^ 1 for i in range(32)]

                def emit_wq_pair(g2):
                    nc.sync.dma_start(
                        out=wq_sb[:, 2 * g2 : 2 * g2 + 2, :],
                        in_=wqd[:, 2 * g2 : 2 * g2 + 2, :],
                    )

                def emit_wkv_quad(g4):
                    sl = slice(4 * g4, 4 * g4 + 4)
                    nc.scalar.dma_start(out=wk_sb[:, sl, :], in_=wkd[:, sl, :])
                    nc.scalar.dma_start(out=wv_sb[:, sl, :], in_=wvd[:, sl, :])

                nc.scalar.dma_start(out=ones_sb[:], in_=onesd[:])
                # warm the ACT exp table during startup DMA
                nc.vector.memset(warm[:], 0.0)
                nc.scalar.activation(out=warm[:], in_=warm[:], func=Exp)

                for n in range(NCH):
                    tsl = bass.ts(n, TCH)
                    # psum: pq0/pq1 double-buffered, rest single
                    pq0 = ppq01.tile([128, TCH], f32, tag="pq0")
                    pq1 = ppq01.tile([128, TCH], f32, tag="pq1")
                    pq2 = ppA.tile([128, TCH], f32, tag="pq2")
                    pq3 = ppA.tile([128, TCH], f32, tag="pq3")
                    pk = ppA.tile([128, TCH], f32, tag="pk")
                    pv = ppA.tile([128, TCH], f32, tag="pv")
                    pq = [pq0, pq1, pq2, pq3]

                    for kp in range(NKC // 2):
                        if n == 0:
                            emit_wq_pair(kp)
                            if kp < 4:
                                emit_wkv_quad(kp)
                        xt = xload.tile([128, 2, TCH], bf16, tag="xt")
                        nc.sync.dma_start(
                            out=xt[:], in_=xd[:, n, 2 * kp : 2 * kp + 2, :]
                        )
                        for u in range(2):
                            k = 2 * kp + u
                            flags = dict(start=(k == 0), stop=(k == NKC - 1))
                            nc.tensor.matmul(
                                pq0[:], wq_sb[:, k, 0:128], xt[:, u, :], **flags
                            )
                            nc.tensor.matmul(
                                pq1[:], wq_sb[:, k, 128:256], xt[:, u, :], **flags
                            )
                            nc.tensor.matmul(pk[:], wk_sb[:, k, :], xt[:, u, :], **flags)
                            nc.tensor.matmul(pv[:], wv_sb[:, k, :], xt[:, u, :], **flags)
                            nc.tensor.matmul(
                                pq2[:], wq_sb[:, k, 256:384], xt[:, u, :], **flags
                            )
                            nc.tensor.matmul(
                                pq3[:], wq_sb[:, k, 384:512], xt[:, u, :], **flags
                            )

                    if n == 0:
                        nc.scalar.dma_start(out=cos_sb[:], in_=cosd[:])
                        nc.scalar.dma_start(out=sin_sb[:], in_=sind[:])

                    # ---- drain PSUM fast across ACT+DVE, then RoPE ----
                    # The head-dim axis is stored host-interleaved (partition
                    # e holds dim (e%2)*64+e//2) so rotate-half is the
                    # adjacent-pair partition swap e^1 — one stream_shuffle.
                    # All rope tensors bf16 for the DVE 2x mode; Pool is 2.7x
                    # slower per element than DVE, so it gets nothing here.
                    vtmp = vt_pool[n % 2]
                    qa_k = qa_pool[0 if n % 2 == 0 else 3]
                    qa_2 = qa_pool[1 if n % 2 == 0 else 4]
                    qa_3 = qa_pool[2 if n % 2 == 0 else 5]
                    nc.scalar.copy(qa_k[:], pk[:])  # ACT: pk, pq3 drains
                    nc.vector.tensor_copy(vtmp[:], pv[:])  # DVE: pv, pq2
                    nc.vector.tensor_copy(qa_2[:], pq2[:])
                    nc.scalar.copy(qa_3[:], pq3[:])
                    nc.sync.dma_start_transpose(v_sb[:, 4 * n : 4 * n + 4, :], vtmp[:])

                    def rope_to(qa, qs, mt, t2, dst):
                        nc.vector.stream_shuffle(qs[:], qa[:], SWAP_MASK)
                        nc.vector.tensor_mul(mt[:], qa[:], cos_sb[:, tsl])
                        nc.vector.tensor_mul(t2[:], qs[:], sin_sb[:, tsl])
                        nc.vector.tensor_add(dst, mt[:], t2[:])

                    # k first (phase B's first need), then q2/q3, then q0/q1
                    m_k, m_2, m_3, m_01 = m_pool
                    t2_k, t2_2, t2_3, t2_01 = t2_pool
                    qs_a, qs_b = qs_pool
                    rope_to(qa_k, qs_a, m_k, t2_k, kT_sb[:, tsl])
                    rope_to(qa_2, qs_b, m_2, t2_2, qT_sb[:, 2, tsl])
                    rope_to(qa_3, qs_a, m_3, t2_3, qT_sb[:, 3, tsl])
                    for j, psrc in ((0, pq0), (1, pq1)):
                        qa = qa_pool[j if n % 2 == 0 else 3 + j]  # reuse slots
                        nc.scalar.copy(qa[:], psrc[:])
                        rope_to(qa, qs_b if j == 0 else qs_a, m_01, t2_01,
                                qT_sb[:, j, tsl])

                # wo is first needed by phase C; the wait_until keeps the
                # scheduler from hoisting its 5.8us transfer into the
                # chunk-critical x/weight DMA stream (observed at t=18us)
                with tc.tile_wait_until(0.080):
                    nc.scalar.dma_start(out=wo_sb[:], in_=wod[:])

            # ---------------- Phase B + C interleaved ----------------
            # B loop is s-chunk outer / head inner so outT for s-chunk sc is
            # complete after 4 blocks; phase C's per-tile matmul chunks are
            # then interleaved into later blocks' PE slack (B is ACT-bound:
            # exp 8.3us vs PE 7.0us per block). y is DMA'd straight from
            # PSUM, so interleaved C adds no ACT work.
            with (
                tc.tile_pool(name="pacc", bufs=2, space="PSUM") as paccp,
                tc.tile_pool(name="prsp", bufs=1, space="PSUM") as prsp,
                tc.tile_pool(name="pyp1", bufs=1, space="PSUM") as pyp1,
            ):
                deferred = [None]
                cqueue = []  # (ready_block_idx, emit_fn) pending C chunks
                cpool = [pyp1]  # swapped to a deeper pool for the tail

                def c_chunk(i, mc, eng=None):
                    def emit():
                        py = cpool[0].tile([128, TCH], f32, tag="py")
                        for kk in range(4):
                            nc.tensor.matmul(
                                py[:],
                                outT_sb[:, kk, bass.ts(i, 128)],
                                wo_sb[:, kk, bass.ts(mc, TCH)],
                                start=(kk == 0),
                                stop=(kk == 3),
                            )
                        # DVE (not ACT) drains py: ACT is phase B's bottleneck
                        ysb = ypool.tile([128, TCH], bf16, tag="ysb")
                        if eng is None:
                            nc.vector.tensor_copy(ysb[:], py[:])
                        else:
                            eng.copy(ysb[:], py[:])
                        nc.sync.dma_start(
                            out=y[bass.ts(i, 128), bass.ts(mc, TCH)], in_=ysb[:]
                        )

                    return emit

                with tc.tile_pool(name="pst", bufs=2, space="PSUM") as pstp:
                    for sc in range(NCH):
                        ssl = bass.ts(sc, TCH)
                        for h in range(4):
                            blk = 4 * sc + h
                            et = epool.tile([128, NT, TCH], bf16, tag="et")
                            pv_acc = paccp.tile([128, TCH], f32, tag="pvacc")

                            def qk_pair(tp):
                                pst = pstp.tile([128, 2, TCH], f32, tag="st")
                                for u in range(2):
                                    tt = 2 * tp + u
                                    nc.tensor.matmul(
                                        pst[:, u, :],
                                        kT_sb[:, bass.ts(tt, 128)],
                                        qT_sb[:, h, ssl],
                                    )
                                nc.scalar.activation(
                                    out=et[:, 2 * tp : 2 * tp + 2, :],
                                    in_=pst[:],
                                    func=Exp,
                                    scale=SCALE,
                                )

                            qk_pair(0)
                            for tp in range(NT // 2):
                                if tp == 3 and deferred[0] is not None:
                                    deferred[0]()
                                    deferred[0] = None
                                if tp in (1, 2, 4, 6) and cqueue and cqueue[0][0] <= blk:
                                    _, ci, cmc = cqueue.pop(0)
                                    c_chunk(ci, cmc)()
                                if tp + 1 < NT // 2:
                                    qk_pair(tp + 1)
                                for u in range(2):
                                    tt = 2 * tp + u
                                    nc.tensor.matmul(
                                        pv_acc[:],
                                        v_sb[:, tt, :],
                                        et[:, tt, :],
                                        start=(tt == 0),
                                        stop=(tt == NT - 1),
                                    )
                                # in-place rowsum tree over the 16 et tiles
                                # (each et[tt] already consumed by PV above):
                                # alternate pair-adds DVE/Pool, upper levels
                                # DVE (bf16 2x: 396ns vs Pool's 1142ns)
                                eng = nc.vector if tp % 2 == 0 else nc.gpsimd
                                eng.tensor_add(
                                    et[:, 2 * tp, :],
                                    et[:, 2 * tp, :],
                                    et[:, 2 * tp + 1, :],
                                )
                                if tp % 2 == 1:
                                    q0 = 4 * (tp // 2)
                                    nc.vector.tensor_add(
                                        et[:, q0, :], et[:, q0, :], et[:, q0 + 2, :]
                                    )
                                if tp == 3:
                                    nc.vector.tensor_add(
                                        et[:, 0, :], et[:, 0, :], et[:, 4, :]
                                    )
                                if tp == 7:
                                    nc.vector.tensor_add(
                                        et[:, 8, :], et[:, 8, :], et[:, 12, :]
                                    )
                                    nc.vector.tensor_add(
                                        et[:, 0, :], et[:, 0, :], et[:, 8, :]
                                    )

                            def make_finish(et=et, pv_acc=pv_acc, h=h, ssl=ssl):
                                def emit():
                                    prs = prsp.tile([128, TCH], f32, tag="prs")
                                    nc.tensor.matmul(prs[:], ones_sb[:], et[:, 0, :])
                                    rec = rpool.tile([128, TCH], f32, tag="rec")
                                    nc.vector.reciprocal(rec[:], prs[:])
                                    nc.vector.tensor_mul(
                                        outT_sb[:, h, ssl], pv_acc[:], rec[:]
                                    )

                                return emit

                            deferred[0] = make_finish()
                            if h == 3:
                                # C chunks for this s-chunk; safe one block
                                # after the deferred normalize lands
                                for i in range(4 * sc, 4 * sc + 4):
                                    for mc in range(NCH):
                                        cqueue.append((blk + 2, i, mc))

                # ---------------- Phase C tail ----------------
                with tc.tile_pool(name="pyp2", bufs=4, space="PSUM") as pyp2:
                    # a couple of sc<3 chunks cover the final normalize's
                    # tree latency, then the rest stream PE-bound; the last
                    # few alternate ACT/DVE drains so the final copies don't
                    # serialize on one engine
                    for _, ci, cmc in cqueue[:2]:
                        c_chunk(ci, cmc)()
                    if deferred[0] is not None:
                        deferred[0]()
                        deferred[0] = None
                    cpool[0] = pyp2
                    rest = cqueue[2:]
                    for idx, (_, ci, cmc) in enumerate(rest):
                        alt = idx >= len(rest) - 6 and idx % 2 == 0
                        c_chunk(ci, cmc, eng=nc.scalar if alt else None)()

    nc.compile()
    return nc


def _rope_tables(start_pos):
    inv = (
        1.0
        / (ROPE_THETA ** (np.arange(0, HD, 2, dtype=np.float32) / np.float32(HD)))
    ).astype(np.float32)
    pos = np.arange(T, dtype=np.float32) + np.float32(start_pos)
    ang = pos[:, None] * inv[None, :]  # [T, 64]
    c = np.cos(ang, dtype=np.float32)
    s = np.sin(ang, dtype=np.float32)
    cosT = np.ascontiguousarray(np.concatenate([c, c], axis=1).T)  # [128, T]
    sin2 = np.concatenate([-s, s], axis=1)  # rotate_half sign folded in
    sinT = np.ascontiguousarray(sin2.T)  # [128, T]
    return cosT, sinT


def kernel(x, Wq, Wk, Wv, Wo, start_pos):
    import os
    import sys

    if os.environ.get("JAX_PLATFORMS") == "cpu" and "jax" not in sys.modules:
        del os.environ["JAX_PLATFORMS"]

    import ml_dtypes

    from concourse.bass_utils import run_bass_kernel_spmd

    bf16 = ml_dtypes.bfloat16

    if "nc" not in _CACHE:
        _CACHE["nc"] = _build_nc()
    nc = _CACHE["nc"]

    x = np.asarray(x, dtype=np.float32)
    Wq = np.asarray(Wq, dtype=np.float32)
    Wk = np.asarray(Wk, dtype=np.float32)
    Wv = np.asarray(Wv, dtype=np.float32)
    Wo = np.asarray(Wo, dtype=np.float32)
    cosT, sinT = _rope_tables(int(start_pos))
    ones = np.ones((128, 128), dtype=bf16)

    # interleaved head-dim order: partition e holds dim (e%2)*64 + e//2, so
    # rotate-half's d <-> d+64 pairing becomes the adjacent swap e <-> e^1
    # (qk dot products are invariant to any shared permutation of d)
    perm = np.array([(e % 2) * 64 + e // 2 for e in range(128)])
    cosT = np.ascontiguousarray(cosT[perm]).astype(bf16)
    sinT = np.ascontiguousarray(sinT[perm]).astype(bf16)

    # host pre-arrangement into exact SBUF layouts (bf16)
    xds = []
    for b in range(B):
        xt = x[b].T  # [C, T]
        xds.append(
            np.ascontiguousarray(
                xt.reshape(NKC, 128, NCH, TCH).transpose(1, 2, 0, 3)
            ).astype(bf16)
        )

    in_maps = []
    for c in range(NCORES):
        b, g = divmod(c, TP)
        wq_g = Wq[512 * g : 512 * (g + 1), :]  # [512, C]
        wq_g = wq_g.reshape(4, 128, C)[:, perm, :].reshape(512, C)
        wk_g = Wk[128 * g : 128 * (g + 1), :][perm, :]  # [128, C]
        wv_g = Wv[128 * g : 128 * (g + 1), :]
        wo_g = Wo[:, 512 * g : 512 * (g + 1)]  # [C, 512]
        in_maps.append(
            {
                "xd": xds[b],
                "wqd": np.ascontiguousarray(
                    wq_g.T.reshape(NKC, 128, 512).transpose(1, 0, 2)
                ).astype(bf16),
                "wkd": np.ascontiguousarray(
                    wk_g.T.reshape(NKC, 128, 128).transpose(1, 0, 2)
                ).astype(bf16),
                "wvd": np.ascontiguousarray(
                    wv_g.T.reshape(NKC, 128, 128).transpose(1, 0, 2)
                ).astype(bf16),
                "wod": np.ascontiguousarray(
                    wo_g.T.reshape(4, 128, C).transpose(1, 0, 2)
                ).astype(bf16),
                "cosd": cosT,
                "sind": sinT,
                "ones": ones,
            }
        )

    _CACHE["in_maps"] = in_maps
    res = run_bass_kernel_spmd(nc, in_maps, list(range(NCORES)))
    out = np.zeros((B, T, C), dtype=np.float32)
    for c in range(NCORES):
        out[c // TP] += np.asarray(res.results[c]["y"], dtype=np.float32)
    return out
